# revision 24
# baseline (speedup 1.0000x reference)
"""Trainium2 Bass kernel for nn_Discriminator (attentional recent discriminator).

Math notes (derived from the module definition, hardcoded here):
  - The attention matmul result is deleted (torch sorts a size-1 dim, so the
    "top-5" indices are always 0); the output depends only on node_vec rows
    0 and N-1 of each batch element.
  - hidden_in rows 1..5 are all node_vec[:,0,:], so the node-MLP only needs
    2 distinct rows per batch; the 6-row structure is recovered in the
    ta_w1 contraction by accumulating the 5 repeated blocks against u0.

Sharding: pure data parallel over batch, 32 batches/core on 8 cores.
Weights + tables replicated. Device does the gathers (indirect DMA),
transposes and matmuls; host shards/reshapes/casts/pads weights and
concatenates outputs.

Gather strategy (indirect DMAs cost ~1.1us each regardless of descriptor
count, and each descriptor serves one dest partition): the 16 small lookups
per node are grouped into 8 PAIRS; a host-built pair table PT[8*200*200, 32]
holds [row16_{2j}(a) | row16_{2j+1}(b)] at row j*40000+200a+b, so one
descriptor fetches two lookups. Pairs 0-3 ride the lower 64 partitions and
pairs 4-7 the upper 64 (trees rows duplicated, upper copy column-shifted by
8), so 4 indirect DMAs of 128 descriptors cover all 1024 lookups, and each
128-partition half of the gather tile transposes into a full K-chunk with a
single PE transpose. Contraction dim = 768 = 16*16 (pe slots zero-padded to
16) + 512 lstm, with node_w1 rows permuted/padded on the host to match.

Precision: bf16 operands with fp32 PSUM accumulation and fp32 biases
(fp32 PE matmuls run as 2 passes; bf16 halves both LDWEIGHTS and MATMUL).
"""

import ml_dtypes
import numpy as np

import concourse.bass as bass
import concourse.mybir as mybir
import concourse.tile as tile
from concourse import bacc
from concourse.bass import IndirectOffsetOnAxis
from concourse.bass_utils import run_bass_kernel_spmd

# problem constants (hardcoded per harness contract)
B, N, M = 256, 200, 200
EMB_DICT, EMB_DIM, POS_DIM, HID, LSTM_DIM, MAX_LEN, TOPK = 200, 16, 8, 32, 512, 200, 5
NODE_DIM = 2 * POS_DIM + 14 * EMB_DIM + LSTM_DIM  # 752

N_CORES = 8
NB = B // N_CORES  # 32 batches per core
NV = 2 * NB        # 64 node vectors per core (node 0 and node N-1)

NSLOT = 16                      # pe0, pe1, emb0..emb13
NPAIR = 8
PAIR_ROWS = EMB_DICT * EMB_DICT  # 40000 rows per pair block
KDIM = 16 * NSLOT + LSTM_DIM    # 768
NCHUNK = KDIM // 128            # 6

F32 = mybir.dt.float32
BF16 = mybir.dt.bfloat16
I32 = mybir.dt.int32
NP_BF16 = ml_dtypes.bfloat16

# wallb (bf16) [128, 1762] column layout
_C_W2 = 768        # rows 0:128
_C_TAW2 = 800      # rows 0:128
_C_TAW1 = 832      # rows 0:32, 6 chunks of 128
_C_FFW2 = 1600     # rows 0:64
_C_FFW1 = 1632     # rows 0:32
_C_TSW = 1696      # rows 0:32
_C_IDENT = 1697    # rows 0:64
_WB_COLS = 1762
# wallf (f32) [128, 7] columns
_F_B1, _F_TAB1, _F_B2, _F_TAB2, _F_FFB1, _F_FFB2, _F_TSB = range(7)


def _pos_encoding():
    pos = np.arange(MAX_LEN, dtype=np.float32)[:, None]
    div = np.exp(
        np.arange(0, POS_DIM, 2, dtype=np.float32) * (-np.log(10000.0) / POS_DIM)
    )
    pe = np.zeros((MAX_LEN, POS_DIM), np.float32)
    pe[:, 0::2] = np.sin(pos * div)
    pe[:, 1::2] = np.cos(pos * div)
    return pe


def build_nc():
    # Bacc (not plain Bass): its compile pass splits multi-wait sync into
    # InstEventSemaphore, which the walrus codegen requires (1 wait/inst).
    nc = bacc.Bacc(
        "TRN2",
        target_bir_lowering=False,
        debug=False,
        num_devices=N_CORES,
    )

    lstm = nc.dram_tensor("lstm", [NB * M, LSTM_DIM], BF16, kind="ExternalInput")
    trees = nc.dram_tensor("trees", [NB, N, 17], I32, kind="ExternalInput")
    pt = nc.dram_tensor("pt", [NPAIR * PAIR_ROWS, 32], BF16, kind="ExternalInput")
    ipack = nc.dram_tensor("ipack", [2 * NV, 5], I32, kind="ExternalInput")
    wallb = nc.dram_tensor("wallb", [128, _WB_COLS], BF16, kind="ExternalInput")
    wallf = nc.dram_tensor("wallf", [128, 7], F32, kind="ExternalInput")

    out_d = nc.dram_tensor("out", [1, NB], F32, kind="ExternalOutput")

    relu = mybir.ActivationFunctionType.Relu
    ident_fn = mybir.ActivationFunctionType.Identity

    with tile.TileContext(nc) as tc:
        with (
            tc.tile_pool(name="sb", bufs=1) as sb,
            tc.tile_pool(name="pst", bufs=2, space="PSUM") as pst,
            tc.tile_pool(name="ps", bufs=1, space="PSUM") as ps,
        ):
            # ---- index data; heads the gather critical path ----
            # trees2: rows 0:64 = (node0 b0..31, nodeL b0..31) cols 0:17;
            #         rows 64:128 = same, columns shifted by 8 (cols 0:9)
            trees2 = sb.tile([2 * NV, 17], I32, tag="trees2")
            ipk = sb.tile([2 * NV, 5], I32, tag="ipk")
            nc.scalar.dma_start(trees2[NV : NV + NB, 0:9], trees[:, 0, 8:17])
            nc.scalar.dma_start(trees2[NV + NB : 2 * NV, 0:9], trees[:, N - 1, 8:17])
            nc.sync.dma_start(trees2[0:NB, :], trees[:, 0, :])
            nc.sync.dma_start(trees2[NB:NV, :], trees[:, N - 1, :])
            nc.scalar.dma_start(ipk[:], ipack[:])

            # lstm flat index (needs lower rows only)
            lidx = sb.tile([NV, 1], I32, tag="lidx")
            nc.vector.tensor_tensor(
                out=lidx[:], in0=trees2[0:NV, 16:17], in1=ipk[0:NV, 4:5],
                op=mybir.AluOpType.add,
            )
            # pair-table indices: 40000*(m + 4*(q//64)) + 200*t_even + t_odd
            pidx = sb.tile([2 * NV, 4], I32, tag="pidx")
            nc.vector.tensor_scalar_mul(pidx[:], trees2[:, 0:8:2], EMB_DICT)
            nc.vector.tensor_tensor(
                out=pidx[:], in0=pidx[:], in1=trees2[:, 1:8:2],
                op=mybir.AluOpType.add,
            )
            nc.vector.tensor_tensor(
                out=pidx[:], in0=pidx[:], in1=ipk[:, 0:4],
                op=mybir.AluOpType.add,
            )

            # ---- weights (second HWDGE queue; transfer overlaps gathers) ----
            wb = sb.tile([128, _WB_COLS], BF16, tag="wb")
            nc.scalar.dma_start(wb[:], wallb[:])
            wf = sb.tile([128, 7], F32, tag="wf")
            nc.sync.dma_start(wf[:], wallf[:])

            # ---- gathers (gpsimd dynamic queue; ~1.1us per instruction) ----
            # order m0, m1, lstm, m2, m3: the first pair half transposes
            # while the lstm rows and second pair half are still in flight.
            # gPair[g] holds pairs {2g, 2g+1}; separate tiles avoid WAR
            # serialization between later gathers and earlier transposes.
            gPair0 = sb.tile([2 * NV, NV], BF16, tag="gPair0")
            gPair1 = sb.tile([2 * NV, NV], BF16, tag="gPair1")
            gPair = [gPair0, gPair1]
            for m in (0, 1):
                nc.gpsimd.indirect_dma_start(
                    out=gPair[0][:, 32 * m : 32 * (m + 1)], out_offset=None,
                    in_=pt[:],
                    in_offset=IndirectOffsetOnAxis(ap=pidx[:, m : m + 1], axis=0),
                )
            nvL = sb.tile([NV, LSTM_DIM], BF16, tag="nvL")
            nc.gpsimd.indirect_dma_start(
                out=nvL[:], out_offset=None, in_=lstm[:],
                in_offset=IndirectOffsetOnAxis(ap=lidx[:, 0:1], axis=0),
            )
            for m in (2, 3):
                nc.gpsimd.indirect_dma_start(
                    out=gPair[1][:, 32 * (m - 2) : 32 * (m - 1)], out_offset=None,
                    in_=pt[:],
                    in_offset=IndirectOffsetOnAxis(ap=pidx[:, m : m + 1], axis=0),
                )

            # ---- transposes into K-major chunks ----
            # pair transpose (h, g): gPair[g] rows 64h (chunk h), K rows
            # 64g:64g+64 of that chunk -> psum rows 64g of ptc_h.
            ident = wb[:NV, _C_IDENT : _C_IDENT + NV]
            vT = sb.tile([128, NCHUNK * NV], BF16, tag="vT")
            ptc0 = ps.tile([128, NV], BF16, tag="ptc0")
            ptc1 = ps.tile([128, NV], BF16, tag="ptc1")
            ptc = [ptc0, ptc1]
            for h in range(2):  # first pair half: chunks 0,1 K-rows 0:64
                nc.tensor.transpose(
                    ptc[h][0:NV, :], gPair[0][NV * h : NV * (h + 1), :],
                    wb[NV * h : NV * (h + 1), _C_IDENT : _C_IDENT + NV],
                )
            for c in range(4):  # lstm chunks 2..5
                ptt = pst.tile([128, NV], BF16, tag="ptt")
                nc.tensor.transpose(
                    ptt[:], nvL[:, 128 * c : 128 * (c + 1)], ident
                )
                nc.vector.tensor_copy(vT[:, bass.ts(2 + c, NV)], ptt[:])
            for h in range(2):  # second pair half: chunks 0,1 K-rows 64:128
                nc.tensor.transpose(
                    ptc[h][NV : 2 * NV, :], gPair[1][NV * h : NV * (h + 1), :],
                    wb[NV * h : NV * (h + 1), _C_IDENT : _C_IDENT + NV],
                )
                nc.vector.tensor_copy(vT[:, bass.ts(h, NV)], ptc[h][:])

            # ---- stage 1: h1T = relu(w1p.T @ vT + b1)  [128, NV] ----
            h1p = ps.tile([128, NV], F32, tag="h1p")
            chunk_order = [2, 3, 4, 5, 0, 1]
            for j, c in enumerate(chunk_order):
                nc.tensor.matmul(
                    h1p[:],
                    lhsT=wb[:, bass.ts(c, 128)],
                    rhs=vT[:, bass.ts(c, NV)],
                    start=(j == 0), stop=(j == NCHUNK - 1),
                )
            h1 = sb.tile([128, NV], BF16, tag="h1")
            nc.scalar.activation(h1[:], h1p[:], relu, bias=wf[:, _F_B1 : _F_B1 + 1])

            # ---- stage 2: uT = relu(w2.T @ h1T + b2)  [32, NV] ----
            up = ps.tile([HID, NV], F32, tag="small_p")
            nc.tensor.matmul(
                up[:], lhsT=wb[:, _C_W2 : _C_W2 + HID], rhs=h1[:],
                start=True, stop=True,
            )
            u = sb.tile([HID, NV], BF16, tag="u")
            nc.scalar.activation(u[:], up[:], relu, bias=wf[:HID, _F_B2 : _F_B2 + 1])

            # ---- stage 3: g1 = relu(sum_i taw1_i.T @ u_sel + tab1)  [128, NB] ----
            # row block 0 of ta_w1 multiplies h(last)=u[:,NB:], blocks 1..5 u[:,:NB]
            g1p = ps.tile([128, NB], F32, tag="mid_p")
            for i in range(6):
                rhs = u[:, NB:NV] if i == 0 else u[:, 0:NB]
                nc.tensor.matmul(
                    g1p[:],
                    lhsT=wb[:HID, _C_TAW1 + 128 * i : _C_TAW1 + 128 * (i + 1)],
                    rhs=rhs,
                    start=(i == 0), stop=(i == 5),
                )
            g1 = sb.tile([128, NB], BF16, tag="g1")
            nc.scalar.activation(
                g1[:], g1p[:], relu, bias=wf[:, _F_TAB1 : _F_TAB1 + 1]
            )

            # ---- stage 4: g2 = relu(taw2.T @ g1 + tab2)  [32, NB] ----
            g2p = ps.tile([HID, NB], F32, tag="small_p")
            nc.tensor.matmul(
                g2p[:], lhsT=wb[:, _C_TAW2 : _C_TAW2 + HID], rhs=g1[:],
                start=True, stop=True,
            )
            g2 = sb.tile([HID, NB], BF16, tag="g2")
            nc.scalar.activation(
                g2[:], g2p[:], relu, bias=wf[:HID, _F_TAB2 : _F_TAB2 + 1]
            )

            # ---- stage 5: g3 = relu(ffw1.T @ g2 + ffb1)  [64, NB] ----
            g3p = ps.tile([2 * HID, NB], F32, tag="mid_p")
            nc.tensor.matmul(
                g3p[:], lhsT=wb[:HID, _C_FFW1 : _C_FFW1 + 2 * HID], rhs=g2[:],
                start=True, stop=True,
            )
            g3 = sb.tile([2 * HID, NB], BF16, tag="g3")
            nc.scalar.activation(
                g3[:], g3p[:], relu, bias=wf[: 2 * HID, _F_FFB1 : _F_FFB1 + 1]
            )

            # ---- stage 6: g4 = relu(ffw2.T @ g3 + ffb2)  [32, NB] ----
            g4p = ps.tile([HID, NB], F32, tag="small_p")
            nc.tensor.matmul(
                g4p[:], lhsT=wb[: 2 * HID, _C_FFW2 : _C_FFW2 + HID], rhs=g3[:],
                start=True, stop=True,
            )
            g4 = sb.tile([HID, NB], BF16, tag="g4")
            nc.scalar.activation(
                g4[:], g4p[:], relu, bias=wf[:HID, _F_FFB2 : _F_FFB2 + 1]
            )

            # ---- stage 7: out = tsw.T @ g4 + tsb  [1, NB] ----
            op = ps.tile([1, NB], F32, tag="small_p")
            nc.tensor.matmul(
                op[:], lhsT=wb[:HID, _C_TSW : _C_TSW + 1], rhs=g4[:],
                start=True, stop=True,
            )
            o = sb.tile([1, NB], F32, tag="o")
            nc.scalar.activation(
                o[:], op[:], ident_fn, bias=wf[:1, _F_TSB : _F_TSB + 1]
            )
            nc.sync.dma_start(out_d[:], o[:])

    nc.finalize()
    return nc


def _slot_rows(inputs):
    """16 lookup tables, each [200, 16] f32 (pe slots zero-padded)."""
    emb = np.asarray(inputs["emb"], np.float32).reshape(EMB_DICT, EMB_DIM)
    pe = _pos_encoding()
    rows = []
    for k in range(2):
        r = np.zeros((EMB_DICT, 16), np.float32)
        r[:, 0:POS_DIM] = pe
        rows.append(r)
    for _ in range(14):
        rows.append(emb)
    return rows


def _pack_pair_table(inputs):
    rows = [r.astype(NP_BF16) for r in _slot_rows(inputs)]
    ptv = np.zeros((NPAIR, EMB_DICT, EMB_DICT, 32), NP_BF16)
    for j in range(NPAIR):
        ptv[j, :, :, 0:16] = rows[2 * j][:, None, :]
        ptv[j, :, :, 16:32] = rows[2 * j + 1][None, :, :]
    return np.ascontiguousarray(ptv.reshape(NPAIR * PAIR_ROWS, 32))


def _pack_weights(inputs):
    def w(name, shape):
        return np.asarray(inputs[name], np.float32).reshape(shape)

    # permute/zero-pad node_w1 rows to the padded 768 contraction order
    w1 = w("node_w1", (NODE_DIM, 4 * HID))
    w1p = np.zeros((KDIM, 4 * HID), np.float32)
    w1p[0:POS_DIM] = w1[0:POS_DIM]                      # slot 0: pe(t0)
    w1p[16 : 16 + POS_DIM] = w1[POS_DIM : 2 * POS_DIM]  # slot 1: pe(t1)
    for j in range(14):                                 # slots 2..15: emb
        w1p[16 * (2 + j) : 16 * (2 + j) + EMB_DIM] = (
            w1[2 * POS_DIM + EMB_DIM * j : 2 * POS_DIM + EMB_DIM * (j + 1)]
        )
    w1p[16 * NSLOT :] = w1[2 * POS_DIM + 14 * EMB_DIM :]  # lstm block

    wb = np.zeros((128, _WB_COLS), np.float32)
    for c in range(NCHUNK):
        wb[:, 128 * c : 128 * (c + 1)] = w1p[128 * c : 128 * (c + 1), :]
    wb[:, _C_W2 : _C_W2 + HID] = w("node_w2", (4 * HID, HID))
    wb[:, _C_TAW2 : _C_TAW2 + HID] = w("ta_w2", (4 * HID, HID))
    taw1 = w("ta_w1", (6 * HID, 4 * HID))
    for i in range(6):
        wb[:HID, _C_TAW1 + 128 * i : _C_TAW1 + 128 * (i + 1)] = (
            taw1[HID * i : HID * (i + 1), :]
        )
    wb[: 2 * HID, _C_FFW2 : _C_FFW2 + HID] = w("ff_w2", (2 * HID, HID))
    wb[:HID, _C_FFW1 : _C_FFW1 + 2 * HID] = w("ff_w1", (HID, 2 * HID))
    wb[:HID, _C_TSW] = w("ts_w", (HID,))
    wb[:, _C_IDENT : _C_IDENT + NV] = np.tile(np.eye(NV, dtype=np.float32), (2, 1))

    wf = np.zeros((128, 7), np.float32)
    wf[:, _F_B1] = w("node_b1", (4 * HID,))
    wf[:, _F_TAB1] = w("ta_b1", (4 * HID,))
    wf[:HID, _F_B2] = w("node_b2", (HID,))
    wf[:HID, _F_TAB2] = w("ta_b2", (HID,))
    wf[: 2 * HID, _F_FFB1] = w("ff_b1", (2 * HID,))
    wf[:HID, _F_FFB2] = w("ff_b2", (HID,))
    wf[0, _F_TSB] = w("ts_b", (1,))[0]
    return wb.astype(NP_BF16), wf


def make_in_maps(inputs):
    lstm = np.asarray(inputs["lstm_out_list"], np.float32).astype(NP_BF16)
    trees = np.ascontiguousarray(np.asarray(inputs["trees"]).astype(np.int32))

    ipack = np.zeros((2 * NV, 5), np.int32)
    ipack[0:NV, 0:4] = np.arange(4, dtype=np.int32)[None, :] * PAIR_ROWS
    ipack[NV :, 0:4] = (np.arange(4, dtype=np.int32)[None, :] + 4) * PAIR_ROWS
    ipack[0:NV, 4] = (np.arange(NV, dtype=np.int32) % NB) * M

    wbv, wfv = _pack_weights(inputs)
    shared = {
        "pt": _pack_pair_table(inputs),
        "ipack": ipack,
        "wallb": wbv,
        "wallf": wfv,
    }
    in_maps = []
    for c in range(N_CORES):
        sl = slice(c * NB, (c + 1) * NB)
        in_maps.append(
            {
                "lstm": np.ascontiguousarray(lstm[sl].reshape(NB * M, LSTM_DIM)),
                "trees": trees[sl],
                **shared,
            }
        )
    return in_maps


_NC_CACHE = None


def run_on_hw(inputs, **kwargs):
    global _NC_CACHE
    if _NC_CACHE is None:
        _NC_CACHE = build_nc()
    in_maps = make_in_maps(inputs)
    return run_bass_kernel_spmd(
        _NC_CACHE, in_maps, core_ids=list(range(N_CORES)), **kwargs
    )


def kernel(**inputs) -> np.ndarray:
    res = run_on_hw(inputs)
    out = np.empty((B, 1), np.float32)
    for c in range(N_CORES):
        out[c * NB : (c + 1) * NB, 0] = res.results[c]["out"][0]
    return out
